# revision 6
# baseline (speedup 1.0000x reference)
"""Trainium2 Bass kernel for GroupedMLP (MoE, expert-parallel over 8 cores).

Math (per expert e, all in bf16, matching the reference):
    fc1   = x_e @ w1_e.T            # [T, 2I]
    inter = fc1[:, :I] * silu(fc1[:, I:])
    out_e = inter @ w2_e.T          # [T, H]

Sharding: expert-parallel -- expert e lives entirely on core e (E == n_cores
== 8).  Every matmul contracts over the partition dimension with zero
on-chip transposes: the host pre-casts to bf16 and pre-transposes each
operand into a "K-inner-partition" HBM layout [128, K/128, free]:

    xk  [128, H/128, T]   = x_e^T     (K = H)
    w1k [128, H/128, 2I]  = w1_e^T    (K = H)
    w2k [128, I/128, H]   = w2_e^T    (K = I)
    ok  [128, H/128, T]   = out_e^T   (device output, un-transposed on host)

On-chip dataflow per core: x^T and inter^T stay resident in SBUF; w1/w2
stream through in 128-column slices; SwiGLU is fused into the PSUM
eviction of the first matmul (ACT does silu, DVE does the multiply).
"""

import numpy as np
import ml_dtypes

import concourse.bass as bass
import concourse.tile as tile
from concourse import bacc, mybir
from concourse.bass_utils import run_bass_kernel_spmd

BF16 = mybir.dt.bfloat16
F32 = mybir.dt.float32
NPBF16 = ml_dtypes.bfloat16

E = 8       # experts == cores
T = 2048    # tokens per expert
H = 2048    # hidden
I = 2048    # intermediate (w1 produces 2*I for swiglu)
P = 128
NSPLIT = 512  # moving free dim per matmul (= one PSUM bank of fp32)


def build_nc(t=T, h=H, i=I, debug=False, use_silu=True, repeats=1):
    """Build the single-core Bass program (run SPMD on all 8 cores).

    use_silu=False replaces the Silu activation with Sigmoid + an extra
    multiply (CoreSim does not implement Silu; hardware does).
    repeats>1 emits the whole MLP body R times back-to-back (same I/O) --
    used only for timing: slope over R isolates steady-state kernel time.
    """
    ks_h = h // P          # k-subtiles contracting over H (matmul 1)
    ks_i = i // P          # k-subtiles contracting over I (matmul 2)
    nq = t // NSPLIT       # moving-dim quarters of the token axis

    nc = bacc.Bacc("TRN2", target_bir_lowering=False, debug=debug,
                   num_devices=E)

    xk = nc.dram_tensor("xk", [P, ks_h, t], BF16, kind="ExternalInput")
    w1k = nc.dram_tensor("w1k", [P, ks_h, 2 * i], BF16, kind="ExternalInput")
    w2k = nc.dram_tensor("w2k", [P, ks_i, h], BF16, kind="ExternalInput")
    ok = nc.dram_tensor("ok", [P, ks_h, t], BF16, kind="ExternalOutput")

    def emit_body(nc, tc, pools):
        xpool, ipool, wpool, tpool, opool, psum = pools
        # Resident tiles, split along tokens so DMAs/deps are per-quarter.
        x_sb = []
        for q in range(nq):
            xt = xpool.tile([P, ks_h, NSPLIT], BF16, name=f"x{q}",
                            tag=f"x{q}", bufs=1)
            nc.sync.dma_start(out=xt[:], in_=xk[:, :, q * NSPLIT:(q + 1) * NSPLIT])
            x_sb.append(xt)
        inter_sb = [
            ipool.tile([P, ks_i, NSPLIT], BF16, name=f"inter{q}",
                       tag=f"inter{q}", bufs=1)
            for q in range(nq)
        ]

        # ---- Phase 1: fc1^T tiles + fused SwiGLU -> inter^T (SBUF) ----
        for it in range(ks_i):
            f0 = it * P
            wa = wpool.tile([P, ks_h, P], BF16, name="wa", tag="wa", bufs=3)
            wb = wpool.tile([P, ks_h, P], BF16, name="wb", tag="wb", bufs=3)
            nc.sync.dma_start(out=wa[:], in_=w1k[:, :, f0:f0 + P])
            nc.sync.dma_start(out=wb[:], in_=w1k[:, :, i + f0:i + f0 + P])
            for q in range(nq):
                pa = psum.tile([P, NSPLIT], F32, name="pa", tag="pa", bufs=2)
                pb = psum.tile([P, NSPLIT], F32, name="pb", tag="pb", bufs=2)
                for ks in range(ks_h):
                    nc.tensor.matmul(
                        pa[:], lhsT=wa[:, ks, :], rhs=x_sb[q][:, ks, :],
                        start=(ks == 0), stop=(ks == ks_h - 1),
                    )
                for ks in range(ks_h):
                    nc.tensor.matmul(
                        pb[:], lhsT=wb[:, ks, :], rhs=x_sb[q][:, ks, :],
                        start=(ks == 0), stop=(ks == ks_h - 1),
                    )
                sl = tpool.tile([P, NSPLIT], F32, name="sl", tag="sl", bufs=3)
                if use_silu:
                    nc.scalar.activation(
                        sl[:], pb[:], mybir.ActivationFunctionType.Silu
                    )
                    nc.vector.tensor_mul(inter_sb[q][:, it, :], pa[:], sl[:])
                else:
                    nc.scalar.activation(
                        sl[:], pb[:], mybir.ActivationFunctionType.Sigmoid
                    )
                    s2 = tpool.tile([P, NSPLIT], F32, name="s2", tag="s2", bufs=3)
                    nc.vector.tensor_mul(s2[:], pb[:], sl[:])
                    nc.vector.tensor_mul(inter_sb[q][:, it, :], pa[:], s2[:])

        # ---- Phase 2: out^T = w2T-slices.T @ inter^T ----
        for ht in range(ks_h):
            w2t = wpool.tile([P, ks_i, P], BF16, name="w2t", tag="w2", bufs=3)
            nc.sync.dma_start(out=w2t[:], in_=w2k[:, :, ht * P:(ht + 1) * P])
            ot = opool.tile([P, t], BF16, name="ot", tag="ot", bufs=2)
            for q in range(nq):
                po = psum.tile([P, NSPLIT], F32, name="po", tag="po", bufs=2)
                for ks in range(ks_i):
                    nc.tensor.matmul(
                        po[:], lhsT=w2t[:, ks, :], rhs=inter_sb[q][:, ks, :],
                        start=(ks == 0), stop=(ks == ks_i - 1),
                    )
                nc.scalar.activation(
                    ot[:, q * NSPLIT:(q + 1) * NSPLIT], po[:],
                    mybir.ActivationFunctionType.Copy,
                )
            nc.sync.dma_start(out=ok[:, ht, :], in_=ot[:])

    with tile.TileContext(nc) as tc:
        with (
            tc.tile_pool(name="xpool", bufs=1) as xpool,
            tc.tile_pool(name="ipool", bufs=1) as ipool,
            tc.tile_pool(name="wpool", bufs=1) as wpool,
            tc.tile_pool(name="tpool", bufs=1) as tpool,
            tc.tile_pool(name="opool", bufs=1) as opool,
            tc.tile_pool(name="psum", bufs=1, space="PSUM") as psum,
        ):
            pools = (xpool, ipool, wpool, tpool, opool, psum)
            for _ in range(repeats):
                emit_body(nc, tc, pools)

    nc.compile()
    return nc


_NC_CACHE = {}


def _get_nc(key=(T, H, I)):
    if key not in _NC_CACHE:
        _NC_CACHE[key] = build_nc(*key)
    return _NC_CACHE[key]


def _shard_host(x, w1, w2, t=T, h=H, i=I):
    """Cast to bf16 and pre-transpose into K-inner-partition layouts."""
    ks_h, ks_i = h // P, i // P
    xb = x.astype(NPBF16)
    w1b = w1.astype(NPBF16)
    w2b = w2.astype(NPBF16)
    in_maps = []
    for e in range(E):
        xe = xb[e * t:(e + 1) * t]                       # [T, H]
        xkk = np.ascontiguousarray(
            xe.T.reshape(ks_h, P, t).transpose(1, 0, 2))  # [P, ks_h, T]
        w1e = w1b[e]                                     # [2I, H]
        w1kk = np.ascontiguousarray(
            w1e.T.reshape(ks_h, P, 2 * i).transpose(1, 0, 2))
        w2e = w2b[e]                                     # [H, I]
        w2kk = np.ascontiguousarray(
            w2e.T.reshape(ks_i, P, h).transpose(1, 0, 2))
        in_maps.append({"xk": xkk, "w1k": w1kk, "w2k": w2kk})
    return in_maps


def _gather_host(results, t=T, h=H):
    out = np.empty((E * t, h), dtype=NPBF16)
    for e in range(E):
        okk = np.asarray(results[e]["ok"])               # [P, ks_h, T]
        out_t = okk.transpose(1, 0, 2).reshape(h, t)     # out^T [H, T]
        out[e * t:(e + 1) * t] = out_t.T
    return out


def run(permuted_local_hidden_states, tokens_per_expert, w1, w2,
        trace=False, **trace_kwargs):
    nc = _get_nc()
    in_maps = _shard_host(permuted_local_hidden_states, w1, w2)
    bkr = run_bass_kernel_spmd(
        nc, in_maps, core_ids=list(range(E)), trace=trace, **trace_kwargs
    )
    return _gather_host(bkr.results), bkr


def kernel(permuted_local_hidden_states, tokens_per_expert, w1, w2):
    out, _ = run(permuted_local_hidden_states, tokens_per_expert, w1, w2)
    return out
